# revision 11
# baseline (speedup 1.0000x reference)
"""Trainium2 Bass kernel: BinarizedLinear  out = x @ (u < weight).T

Shapes (hardcoded): x [16384, 4096] f32, weight/u [512, 4096] f32,
out [16384, 512] f32.

Sharding: data-parallel over 8 NeuronCores — x sharded along batch
(2048 rows/core), weight/u replicated, no collectives; host concatenates
the per-core outputs.

Per-core kernel (Tile framework):
  Phase A: load weight/u (fp32), binarize on DVE (u < weight -> bf16
           {0,1}), xbar-DMA-transpose to wbt[i_local, k, o] so the
           contraction dim (INUM) is on partitions, then (fp8 path)
           cast bf16 -> fp8e4. wbt stays resident in SBUF.
  Phase B: per 128-row batch tile: SWDGE cast-load x fp32->bf16,
           xbar-DMA-transpose to xt[i_local, k, b_local], (fp8 path)
           cast bf16 -> fp8e4 split across DVE+ACT, then accumulating
           PE matmuls per output tile [128 b, 512 o] (fp8e4 DoubleRow:
           two k-tiles per instruction at 0.5 cycles/row, fp32 PSUM),
           DVE copy PSUM->SBUF, store.

fp8e4 ({0,1} weights are exact; x in [0,1) quantizes at ~3% RMS per
element, averaging out to ~1e-3 relative on the 4096-term dot product,
well inside the 2e-2 gate) doubles PE matmul throughput vs bf16.
"""

import numpy as np

from concourse import bass, bacc, mybir, tile
from concourse.bass_utils import run_bass_kernel_spmd

B, INUM, ONUM = 16384, 4096, 512
NCORES = 8
BLOC = B // NCORES  # 2048 batch rows per core
P = 128             # partitions
NK = INUM // P      # 32 contraction tiles
NOT = ONUM // P     # 4 weight-row tiles

F32 = mybir.dt.float32
BF16 = mybir.dt.bfloat16
F8 = mybir.dt.float8e4

_CACHE = {}


def build(bloc=BLOC, gb=2, xn_bufs=3, xt_bufs=4, ob_bufs=4, ps_bufs=8,
          store_gb=1, loop=None, pe_groups=(), fp8=True, mode="full"):
    """gb: batch tiles (of 128 rows) grouped per x load/transpose DMA.

    loop: if set, wrap phase B in a For_i repeating it `loop` times
    (timing variant: same data each iteration, outputs overwritten).

    pe_groups: group indices whose x-transpose runs on the tensor engine
    (identity matmul -> bf16 PSUM -> DVE copy) instead of the DMA xbar,
    rebalancing SDMA-engine work onto PE idle time.

    fp8: 2-byte-unit xbar transpose of fp8 data + DoubleRowSwInterleave
    matmuls (2 k-tiles per instruction); output batch tiles come back
    row-reversed (host un-reverses).

    mode: timing-variant scopes for component benches — "full" (default),
    "dma" (x loads + transposes only), "pe" (matmuls/copies/stores from
    one resident xt tile, no per-pass DMA in the loop)."""
    nbt = bloc // P
    ngrp = nbt // gb
    NKP = INUM // 256    # fp8 paired k'-tiles (256 contraction per matmul)
    nc = bacc.Bacc("TRN2", target_bir_lowering=False, debug=False,
                   num_devices=NCORES)
    x_d = nc.dram_tensor("x", [bloc, INUM], F32, kind="ExternalInput")
    w_d = nc.dram_tensor("weight", [ONUM, INUM], F32, kind="ExternalInput")
    u_d = nc.dram_tensor("u", [ONUM, INUM], F32, kind="ExternalInput")
    o_d = nc.dram_tensor("out", [bloc, ONUM], F32, kind="ExternalOutput")

    # DRAM views, partition-major: x_v[g][p, j, i] = x[(g*gb + j)*P + p, i]
    x_v = x_d[:, :].rearrange("(g j p) i -> g p j i", g=ngrp, j=gb, p=P)
    o_v = o_d[:, :].rearrange("(g j p) o -> g p j o", g=nbt // store_gb,
                              j=store_gb, p=P)

    if pe_groups:
        ps_bufs = min(ps_bufs, 6)   # leave 2 PSUM banks for PE transposes

    with tile.TileContext(nc) as tc:
        with (
            tc.tile_pool(name="wbt", bufs=1) as wbt_pool,
            tc.tile_pool(name="ps", bufs=ps_bufs, space="PSUM") as ps_pool,
        ):
            from contextlib import ExitStack
            _aux = ExitStack()
            ident = None
            if pe_groups:
                from concourse import masks
                ident_pool = _aux.enter_context(
                    tc.tile_pool(name="ident", bufs=1))
                ident = ident_pool.tile([P, P], BF16)
                masks.make_identity(nc, ident[:])
            # ---- Phase A: binarized, transposed weights (resident) ----
            # bf16 path: wbt[i_local, k, o] = (u < weight)[o, k*128 + i]
            # fp8 path:  the binarized row is cast to fp8e4 and the xbar
            #   transpose runs on a bf16 *view*, so each transposed 2-byte
            #   unit carries the (i=2q, i=2q+1) fp8 pair; the pair becomes
            #   DoubleRow's two k-planes via byte-strided APs. wbt8_bf
            #   [u, k', o] (bf16 units) = fp8 bytes wb[o, k'*256 + 2u + e].
            # One 2MB DMA per o-tile per tensor (chunking these into 32
            # small DMAs put ~80us of serialized SWDGE latency on the
            # critical path gating the first matmul); w rides SWDGE while
            # u rides the scalar HWDGE so descriptor generation overlaps.
            # wu/wb pools are phase-A-scoped; SBUF is reclaimed for B.
            if fp8:
                wbt8_bf = wbt_pool.tile([P, NKP, ONUM], BF16)
                wbt8_f8 = wbt8_bf[:].bitcast(F8)   # [P, NKP, 2*ONUM]
            else:
                wbt = wbt_pool.tile([P, NK, ONUM], BF16)
            with (
                tc.tile_pool(name="wu", bufs=2) as wu_pool,
                tc.tile_pool(name="wb", bufs=2) as wb_pool,
            ):
                for ot in range(NOT):
                    w_t = wu_pool.tile([P, INUM], F32, tag="w")
                    u_t = wu_pool.tile([P, INUM], F32, tag="u")
                    nc.gpsimd.dma_start(out=w_t[:],
                                        in_=w_d[ot * P:(ot + 1) * P, :])
                    nc.scalar.dma_start(out=u_t[:],
                                        in_=u_d[ot * P:(ot + 1) * P, :])
                    if fp8:
                        wb8_t = wb_pool.tile([P, INUM], F8, tag="wb8")
                        nc.vector.tensor_tensor(wb8_t[:], u_t[:], w_t[:],
                                                op=mybir.AluOpType.is_lt)
                        nc.sync.dma_start(
                            out=wbt8_bf[:, :, ot * P:(ot + 1) * P],
                            in_=wb8_t[:].bitcast(BF16), transpose=True)
                    else:
                        wb_t = wb_pool.tile([P, INUM], BF16, tag="wb")
                        nc.vector.tensor_tensor(wb_t[:], u_t[:], w_t[:],
                                                op=mybir.AluOpType.is_lt)
                        nc.sync.dma_start(
                            out=wbt[:, :, ot * P:(ot + 1) * P],
                            in_=wb_t[:], transpose=True)

            # ---- Phase B: stream batch tiles, gb tiles per DMA group ----
            with (
                tc.tile_pool(name="xn", bufs=xn_bufs) as xn_pool,
                tc.tile_pool(name="xt", bufs=xt_bufs) as xt_pool,
                tc.tile_pool(name="ob", bufs=ob_bufs) as ob_pool,
            ):
                pst_pool = None
                if pe_groups:
                    pst_pool = _aux.enter_context(
                        tc.tile_pool(name="pst", bufs=2, space="PSUM"))
                QT = 4   # k-tiles per PE-transpose PSUM staging quad

                def run_groups(_iv=None, prefetch=2):
                    # software-pipelined emission: group g+prefetch's
                    # load+transpose are emitted before group g's matmuls,
                    # so the prefetch DMAs outrank compute in the
                    # scheduler's priority order
                    xts = {}
                    if mode == "pe":
                        run_pe_only()
                        return

                    def emit_fetch(g):
                        if fp8:
                            # xn8[p, j, i] = fp8(x[(g*gb + j)*P + p, i]),
                            # cast in the DMA; the xbar transpose then moves
                            # 2-byte units, i.e. (2q, 2q+1) fp8 pairs:
                            # xt8_bf[u, j*NKP + t, b] unit = fp8 bytes
                            # x[(g*gb+j)*P + b, t*256 + 2u + e]
                            xn8 = xn_pool.tile([P, gb, INUM], F8, tag="xn")
                            nc.gpsimd.dma_start(out=xn8[:], in_=x_v[g])
                            xt = xt_pool.tile([P, gb * NKP, P], BF16,
                                              tag="xt")
                            nc.sync.dma_start(out=xt[:],
                                              in_=xn8[:].bitcast(BF16),
                                              transpose=True)
                        else:
                            # xn[p, j, i] = x[(g*gb + j)*P + p, i]
                            xn = xn_pool.tile([P, gb, INUM], BF16, tag="xn")
                            nc.gpsimd.dma_start(out=xn[:], in_=x_v[g])
                            # xt[p, j*NK + k, f] = xn_2d[f, j*INUM + k*P + p]
                            #                    = x[(g*gb + j)*P + f, k*P+p]
                            xt = xt_pool.tile([P, gb * NK, P], BF16,
                                              tag="xt")
                            emit_transpose(g, xn, xt)
                        xts[g] = xt

                    ob = None
                    if mode == "dma":
                        for g in range(ngrp):
                            emit_fetch(g)
                        return
                    for g in range(min(prefetch + 1, ngrp)):
                        emit_fetch(g)
                    for g in range(ngrp):
                        xt = xts.pop(g)
                        emit_compute(xt, g)
                        if g + prefetch + 1 < ngrp:
                            emit_fetch(g + prefetch + 1)

                def run_pe_only():
                    for g in range(ngrp):
                        emit_compute(xt_res, g)

                def emit_compute(xt, g):
                    ob = None
                    if True:
                        for j in range(gb):
                            bt = g * gb + j
                            jj = bt % store_gb
                            if jj == 0:
                                ob = ob_pool.tile([P, store_gb, ONUM], F32,
                                                  tag="ob")
                            ps = ps_pool.tile([P, ONUM], F32, tag="ps")
                            if fp8:
                                # DoubleRowSwInterleave: the stationary
                                # operand is the RAW pair-interleaved 256-byte
                                # block the 2-byte xbar transpose produced
                                # (walrus rejects byte-strided Ldweights APs;
                                # SWI is the hw mode for interleaved weights).
                                # The hw reads weight columns last-first, so
                                # psum partition m holds batch row 127-m; the
                                # host un-reverses each 128-row block.
                                xt_f8 = xt[:].bitcast(F8)  # [P, gb*NKP, 256]
                                for t in range(NKP):
                                    lhsT = xt_f8[:, j * NKP + t, :].rearrange(
                                        "p (a b) -> p a b", a=2)
                                    rhs = wbt8_f8[:, t, :].rearrange(
                                        "p (o e) -> p e o", e=2)
                                    nc.tensor.matmul(
                                        ps[:], lhsT, rhs,
                                        start=(t == 0), stop=(t == NKP - 1),
                                        perf_mode=mybir.MatmulPerfMode
                                        .DoubleRowSwInterleave)
                            else:
                                for k in range(NK):
                                    nc.tensor.matmul(ps[:],
                                                     xt[:, j * NK + k, :],
                                                     wbt[:, k, :],
                                                     start=(k == 0),
                                                     stop=(k == NK - 1))
                            nc.vector.tensor_copy(ob[:, jj, :], ps[:])
                            if jj == store_gb - 1:
                                nc.scalar.dma_start(out=o_v[bt // store_gb],
                                                    in_=ob[:])
                        if g + prefetch + 1 < ngrp:
                            emit_fetch(g + prefetch + 1)

                def emit_transpose(g, xn, xt):
                    if g in pe_groups:
                        # tensor-engine transpose: each [128b,128i]
                        # sub-tile via identity matmul into bf16 PSUM,
                        # then DVE copies a quad back to SBUF
                        for j in range(gb):
                            for q in range(NK // QT):
                                pst = pst_pool.tile([P, QT, P], BF16,
                                                    tag="pst")
                                for h in range(QT):
                                    k = q * QT + h
                                    nc.tensor.transpose(
                                        pst[:, h, :],
                                        xn[:, j, k * P:(k + 1) * P],
                                        ident[:])
                                kk = j * NK + q * QT
                                nc.vector.tensor_copy(
                                    xt[:, kk:kk + QT, :], pst[:])
                    else:
                        nc.sync.dma_start(out=xt[:], in_=xn[:],
                                          transpose=True)

                if loop is None:
                    run_groups()
                else:
                    with tc.For_i(0, loop, 1):
                        run_groups()
            _aux.close()   # LIFO: release pst/ident after xn/xt/ob

    nc.compile()
    return nc


def _make_exec(nc):
    """Build a jitted shard_map executable over the 8 cores (mirrors
    bass2jax.run_bass_via_pjrt's multi-core path, without donation so the
    same device buffers can be re-executed for timing)."""
    import jax
    from jax.sharding import Mesh, PartitionSpec
    from jax.experimental.shard_map import shard_map
    from concourse import bass2jax

    bass2jax.install_neuronx_cc_hook()
    partition_name = (nc.partition_id_tensor.name
                      if nc.partition_id_tensor else None)
    in_names, out_names, out_avals = [], [], []
    for alloc in nc.m.functions[0].allocations:
        if not isinstance(alloc, mybir.MemoryLocationSet):
            continue
        name = alloc.memorylocations[0].name
        if alloc.kind == "ExternalInput":
            if name != partition_name:
                in_names.append(name)
        elif alloc.kind == "ExternalOutput":
            out_names.append(name)
            out_avals.append(jax.core.ShapedArray(
                tuple(alloc.tensor_shape), mybir.dt.np(alloc.dtype)))
    n_params = len(in_names)
    all_names = in_names + out_names
    if partition_name is not None:
        all_names = all_names + [partition_name]

    def _body(*args):
        operands = list(args)
        if partition_name is not None:
            operands.append(bass2jax.partition_id_tensor())
        return tuple(bass2jax._bass_exec_p.bind(
            *operands,
            out_avals=tuple(out_avals),
            in_names=tuple(all_names),
            out_names=tuple(out_names),
            lowering_input_output_aliases=(),
            sim_require_finite=True,
            sim_require_nnan=True,
            nc=nc,
        ))

    devices = jax.devices()[:NCORES]
    mesh = Mesh(np.asarray(devices), ("core",))

    def make_fn(reps):
        def _rep_body(*args):
            outs = None
            for _ in range(reps):
                outs = _body(*args)   # effectful primitive: not CSE'd
            return outs
        return jax.jit(
            shard_map(_rep_body, mesh=mesh,
                      in_specs=(PartitionSpec("core"),) * (n_params + len(out_names)),
                      out_specs=(PartitionSpec("core"),) * len(out_names),
                      check_rep=False),
            keep_unused=True,
        )

    return make_fn, mesh, in_names[:n_params], out_names, out_avals


def bench(x, weight, u, r_lo=32, r_hi=1024, iters=6, **build_kw):
    """Measure real device time for one kernel execution.

    The axon RPC jitter (tens of ms) swamps a single ~250us execution, and
    multiple identical bass_exec calls in one program get CSE'd. So we
    build two NEFF variants whose phase B repeats in an on-device For_i
    loop (r_lo and r_hi iterations) and difference the wall-clock minima:
    (t_hi - t_lo)/(r_hi - r_lo) is one full phase-B pass of device time.
    Phase A (binarize+transpose weights, ~25us, runs once) is added from
    its cost-model share."""
    import time
    import jax
    from jax.sharding import NamedSharding, PartitionSpec

    concat = {
        "x": np.ascontiguousarray(x, dtype=np.float32),
        "weight": np.concatenate([weight] * NCORES, axis=0),
        "u": np.concatenate([u] * NCORES, axis=0),
    }

    def run_variant(r):
        nc = build(loop=r, **build_kw)
        make_fn, mesh, in_names, out_names, out_avals = _make_exec(nc)
        sh = NamedSharding(mesh, PartitionSpec("core"))
        args = [jax.device_put(concat[n], sh) for n in in_names]
        zeros = [
            jax.device_put(
                np.zeros((NCORES * a.shape[0], *a.shape[1:]), a.dtype), sh)
            for a in out_avals
        ]
        fn = make_fn(1)
        jax.block_until_ready(fn(*args, *zeros))    # compile + warm
        best = float("inf")
        for _ in range(iters):
            t0 = time.perf_counter()
            jax.block_until_ready(fn(*args, *zeros))
            best = min(best, time.perf_counter() - t0)
        return best

    t_lo = run_variant(r_lo)
    t_hi = run_variant(r_hi)
    pass_ns = (t_hi - t_lo) / (r_hi - r_lo) * 1e9
    phase_a_ns = 25_000.0   # one-time weight binarize+transpose (cost model)
    print(f"bench: loop{r_lo}={t_lo*1e3:.1f}ms loop{r_hi}={t_hi*1e3:.1f}ms "
          f"-> phase-B pass {pass_ns/1e3:.1f}us + phase-A ~{phase_a_ns/1e3:.0f}us")
    return pass_ns + phase_a_ns


def kernel(x, weight, u):
    x = np.ascontiguousarray(np.asarray(x), dtype=np.float32)
    weight = np.ascontiguousarray(np.asarray(weight), dtype=np.float32)
    u = np.ascontiguousarray(np.asarray(u), dtype=np.float32)
    assert x.shape == (B, INUM) and weight.shape == (ONUM, INUM)

    nc = _CACHE.get("nc")
    if nc is None:
        nc = _CACHE["nc"] = build()

    in_maps = [
        {"x": x[c * BLOC:(c + 1) * BLOC], "weight": weight, "u": u}
        for c in range(NCORES)
    ]
    res = run_bass_kernel_spmd(nc, in_maps, list(range(NCORES)))
    out = np.concatenate([res.results[c]["out"] for c in range(NCORES)],
                         axis=0)
    # SWI matmuls emit each 128-row batch tile with rows reversed
    out = np.ascontiguousarray(
        out.reshape(-1, P, ONUM)[:, ::-1, :].reshape(B, ONUM))
    return out



# revision 33
# speedup vs baseline: 1.3828x; 1.3828x over previous
"""Trainium2 Bass kernel: BinarizedLinear  out = x @ (u < weight).T

Shapes (hardcoded): x [16384, 4096] f32, weight/u [512, 4096] f32,
out [16384, 512] f32.

Sharding: data-parallel over 8 NeuronCores — x sharded along batch
(2048 rows/core), weight/u replicated, no collectives; host concatenates
the per-core outputs.

Per-core kernel (Tile framework), default pe_t=True design:
  Phase A: load weight/u (fp32), binarize on DVE (u < weight -> bf16
           {0,1}), xbar-DMA-transpose so the contraction dim lands on
           partitions, cast to plane-packed fp8e4 wbt8p[i_loc, k, o]
           (resident, 2 MB).
  Phase B: per 2-batch-tile group: SWDGE cast-load x fp32->fp8e4
           (read-side HBM bound, ~330 GB/s/core), transpose on the
           TENSOR engine (identity matmul -> fp8 PSUM -> DVE/ACT copy
           to SBUF) keeping the shared DMA bus free of xbar traffic,
           then fp8 DoubleRow matmuls (two 128-k-tiles per instruction,
           fp32 PSUM), DVE copy PSUM->SBUF, store.

The alternate pe_t=False path transposes fp8 data on the DMA xbar in
2-byte units (a (2q,2q+1) fp8 pair per unit) and consumes the
pair-interleaved block directly as DoubleRowSwInterleave stationary
operands; the hw reads weight columns last-first, so each 128-row
batch tile returns row-reversed and the host unreverses. Measured on
hw: the xbar path spends ~62us/pass of shared-bus time on transposes,
the pe_t path moves that to PE idle time.

fp8e4 ({0,1} weights are exact; x in [0,1) quantizes at ~3% RMS per
element, averaging out to ~6e-4 relative on the 4096-term dot product,
well inside the 2e-2 gate) doubles PE matmul throughput vs bf16.
"""

import numpy as np

from concourse import bass, bacc, mybir, tile
from concourse.bass_utils import run_bass_kernel_spmd

B, INUM, ONUM = 16384, 4096, 512
NCORES = 8
BLOC = B // NCORES  # 2048 batch rows per core
P = 128             # partitions
NK = INUM // P      # 32 contraction tiles
NOT = ONUM // P     # 4 weight-row tiles

F32 = mybir.dt.float32
BF16 = mybir.dt.bfloat16
F8 = mybir.dt.float8e4

# default transpose strategy: pe_t=True moves the x transposes off the
# shared DMA bus onto the tensor engine (plain DoubleRow, no row
# reversal); pe_t=False is the xbar/SwInterleave path whose 128-row
# batch tiles come back row-reversed (host unreverses).
DEFAULT_PE_T = True

_CACHE = {}


def build(bloc=BLOC, gb=2, xn_bufs=3, xt_bufs=4, ob_bufs=4, ps_bufs=8,
          store_gb=1, loop=None, pe_groups=(), fp8=True, mode="full",
          pe_t=None, split_q=False):
    """gb: batch tiles (of 128 rows) grouped per x load/transpose DMA.

    loop: if set, wrap phase B in a For_i repeating it `loop` times
    (timing variant: same data each iteration, outputs overwritten).

    pe_groups: group indices whose x-transpose runs on the tensor engine
    (identity matmul -> bf16 PSUM -> DVE copy) instead of the DMA xbar,
    rebalancing SDMA-engine work onto PE idle time.

    fp8: 2-byte-unit xbar transpose of fp8 data + DoubleRowSwInterleave
    matmuls (2 k-tiles per instruction); output batch tiles come back
    row-reversed (host un-reverses).

    mode: timing-variant scopes for component benches — "full" (default),
    "dma" (x loads + transposes only), "pe" (matmuls/copies/stores from
    one resident xt tile, no per-pass DMA in the loop)."""
    if pe_t is None:
        pe_t = DEFAULT_PE_T
    nbt = bloc // P
    ngrp = nbt // gb
    NKP = INUM // 256    # fp8 paired k'-tiles (256 contraction per matmul)
    nc = bacc.Bacc("TRN2", target_bir_lowering=False, debug=False,
                   num_devices=NCORES)
    x_d = nc.dram_tensor("x", [bloc, INUM], F32, kind="ExternalInput")
    w_d = nc.dram_tensor("weight", [ONUM, INUM], F32, kind="ExternalInput")
    u_d = nc.dram_tensor("u", [ONUM, INUM], F32, kind="ExternalInput")
    o_d = nc.dram_tensor("out", [bloc, ONUM], F32, kind="ExternalOutput")

    # DRAM views, partition-major: x_v[g][p, j, i] = x[(g*gb + j)*P + p, i]
    x_v = x_d[:, :].rearrange("(g j p) i -> g p j i", g=ngrp, j=gb, p=P)
    o_v = o_d[:, :].rearrange("(g j p) o -> g p j o", g=nbt // store_gb,
                              j=store_gb, p=P)

    if pe_groups or pe_t:
        ps_bufs = min(ps_bufs, 6)   # leave 2 PSUM banks for PE transposes

    with tile.TileContext(nc) as tc:
        with (
            tc.tile_pool(name="wbt", bufs=1) as wbt_pool,
            tc.tile_pool(name="ps", bufs=ps_bufs, space="PSUM") as ps_pool,
        ):
            from contextlib import ExitStack
            _aux = ExitStack()
            ident = ident8 = None
            if pe_groups:
                from concourse import masks
                ident_pool = _aux.enter_context(
                    tc.tile_pool(name="ident", bufs=1))
                ident = ident_pool.tile([P, P], BF16)
                masks.make_identity(nc, ident[:])
            if fp8 and pe_t:
                from concourse import masks
                ident_pool = _aux.enter_context(
                    tc.tile_pool(name="ident8", bufs=1))
                ident_b = ident_pool.tile([P, P], BF16)
                masks.make_identity(nc, ident_b[:])
                ident8 = ident_pool.tile([P, P], F8)
                nc.vector.tensor_copy(ident8[:], ident_b[:])
            # ---- Phase A: binarized, transposed weights (resident) ----
            # bf16 path: wbt[i_local, k, o] = (u < weight)[o, k*128 + i]
            # fp8 path:  the binarized row is cast to fp8e4 and the xbar
            #   transpose runs on a bf16 *view*, so each transposed 2-byte
            #   unit carries the (i=2q, i=2q+1) fp8 pair; the pair becomes
            #   DoubleRow's two k-planes via byte-strided APs. wbt8_bf
            #   [u, k', o] (bf16 units) = fp8 bytes wb[o, k'*256 + 2u + e].
            # One 2MB DMA per o-tile per tensor (chunking these into 32
            # small DMAs put ~80us of serialized SWDGE latency on the
            # critical path gating the first matmul); w rides SWDGE while
            # u rides the scalar HWDGE so descriptor generation overlaps.
            # wu/wb pools are phase-A-scoped; SBUF is reclaimed for B.
            if fp8 and pe_t:
                # plane-packed fp8 weights for plain DoubleRow:
                # wbt8p[i_loc, k, o] = fp8(wb[o, k*128 + i_loc])
                wbt8p = wbt_pool.tile([P, NK, ONUM], F8)
            elif fp8:
                wbt8_bf = wbt_pool.tile([P, NKP, ONUM], BF16)
                wbt8_f8 = wbt8_bf[:].bitcast(F8)   # [P, NKP, 2*ONUM]
            else:
                wbt = wbt_pool.tile([P, NK, ONUM], BF16)
            with (
                tc.tile_pool(name="wu", bufs=2) as wu_pool,
                tc.tile_pool(name="wb", bufs=2) as wb_pool,
            ):
                for ot in range(NOT):
                    w_t = wu_pool.tile([P, INUM], F32, tag="w")
                    u_t = wu_pool.tile([P, INUM], F32, tag="u")
                    nc.gpsimd.dma_start(out=w_t[:],
                                        in_=w_d[ot * P:(ot + 1) * P, :])
                    nc.scalar.dma_start(out=u_t[:],
                                        in_=u_d[ot * P:(ot + 1) * P, :])
                    if fp8 and pe_t:
                        wb_t = wb_pool.tile([P, INUM], BF16, tag="wb")
                        nc.vector.tensor_tensor(wb_t[:], u_t[:], w_t[:],
                                                op=mybir.AluOpType.is_lt)
                        wt_t = wb_pool.tile([P, NK, P], BF16, tag="wt")
                        nc.sync.dma_start(out=wt_t[:], in_=wb_t[:],
                                          transpose=True)
                        cast = (nc.vector.tensor_copy if ot % 2 == 0
                                else nc.scalar.copy)
                        cast(wbt8p[:, :, ot * P:(ot + 1) * P], wt_t[:])
                    elif fp8:
                        wb8_t = wb_pool.tile([P, INUM], F8, tag="wb8")
                        nc.vector.tensor_tensor(wb8_t[:], u_t[:], w_t[:],
                                                op=mybir.AluOpType.is_lt)
                        nc.sync.dma_start(
                            out=wbt8_bf[:, :, ot * P:(ot + 1) * P],
                            in_=wb8_t[:].bitcast(BF16), transpose=True)
                    else:
                        wb_t = wb_pool.tile([P, INUM], BF16, tag="wb")
                        nc.vector.tensor_tensor(wb_t[:], u_t[:], w_t[:],
                                                op=mybir.AluOpType.is_lt)
                        nc.sync.dma_start(
                            out=wbt[:, :, ot * P:(ot + 1) * P],
                            in_=wb_t[:], transpose=True)

            # ---- Phase B: stream batch tiles, gb tiles per DMA group ----
            with (
                tc.tile_pool(name="xn", bufs=xn_bufs) as xn_pool,
                tc.tile_pool(name="xt", bufs=xt_bufs) as xt_pool,
                tc.tile_pool(name="ob", bufs=ob_bufs) as ob_pool,
            ):
                pst_pool = None
                if pe_groups or (fp8 and pe_t):
                    pst_pool = _aux.enter_context(
                        tc.tile_pool(name="pst", bufs=2, space="PSUM"))
                QT = 4   # k-tiles per PE-transpose PSUM staging quad

                def emit_load(xn8, g):
                    # casting DMAs are SWDGE-only, so the fp8 cast-load
                    # always rides the gpsimd queue
                    nc.gpsimd.dma_start(out=xn8[:], in_=x_v[g])

                def run_groups(_iv=None, prefetch=2):
                    # software-pipelined emission: group g+prefetch's
                    # load+transpose are emitted before group g's matmuls,
                    # so the prefetch DMAs outrank compute in the
                    # scheduler's priority order
                    xts = {}
                    if mode == "pe":
                        run_pe_only()
                        return

                    def emit_fetch(g):
                        if fp8 and pe_t:
                            # fp8 cast-load, then elementwise PE transposes
                            # (identity matmul, fp8 PSUM) + DVE/ACT copies
                            # to plane-packed xt8 — keeps the shared DMA bus
                            # free of the 2x xbar traffic.
                            xn8 = xn_pool.tile([P, gb, INUM], F8, tag="xn")
                            emit_load(xn8, g)
                            xt = xt_pool.tile([P, gb * NK, P], F8, tag="xt")
                            for j in range(gb):
                                for q in range(NK // QT):
                                    # fp8 PE transpose writes PSUM with an
                                    # element step of 2 bytes: stage in a
                                    # bf16 tile addressed via a stride-2
                                    # fp8 view
                                    pst = pst_pool.tile([P, QT, P], BF16,
                                                        tag="pst")
                                    pv = pst[:].bitcast(F8).rearrange(
                                        "p q (b e) -> p q e b", e=2)
                                    for h in range(QT):
                                        k = q * QT + h
                                        nc.tensor.transpose(
                                            pv[:, h, 0, :],
                                            xn8[:, j, k * P:(k + 1) * P],
                                            ident8[:])
                                    kk = j * NK + q * QT
                                    copy = (nc.vector.tensor_copy
                                            if q % 2 == 0 else nc.scalar.copy)
                                    copy(xt[:, kk:kk + QT, :], pv[:, :, 0, :])
                        elif fp8:
                            # xn8[p, j, i] = fp8(x[(g*gb + j)*P + p, i]),
                            # cast in the DMA; the xbar transpose then moves
                            # 2-byte units, i.e. (2q, 2q+1) fp8 pairs:
                            # xt8_bf[u, j*NKP + t, b] unit = fp8 bytes
                            # x[(g*gb+j)*P + b, t*256 + 2u + e]
                            xn8 = xn_pool.tile([P, gb, INUM], F8, tag="xn")
                            nc.gpsimd.dma_start(out=xn8[:], in_=x_v[g])
                            xt = xt_pool.tile([P, gb * NKP, P], BF16,
                                              tag="xt")
                            nc.sync.dma_start(out=xt[:],
                                              in_=xn8[:].bitcast(BF16),
                                              transpose=True)
                        else:
                            # xn[p, j, i] = x[(g*gb + j)*P + p, i]
                            xn = xn_pool.tile([P, gb, INUM], BF16, tag="xn")
                            nc.gpsimd.dma_start(out=xn[:], in_=x_v[g])
                            # xt[p, j*NK + k, f] = xn_2d[f, j*INUM + k*P + p]
                            #                    = x[(g*gb + j)*P + f, k*P+p]
                            xt = xt_pool.tile([P, gb * NK, P], BF16,
                                              tag="xt")
                            emit_transpose(g, xn, xt)
                        xts[g] = xt

                    ob = None
                    if mode == "dma":
                        for g in range(ngrp):
                            emit_fetch(g)
                        return
                    if mode == "load":
                        for g in range(ngrp):
                            xn8 = xn_pool.tile([P, gb, INUM],
                                               F8 if fp8 else BF16, tag="xn")
                            emit_load(xn8, g)
                        return
                    if mode == "loadraw":
                        # raw fp32 loads (no cast) split across two HWDGE
                        # queues — probes whether the load ceiling is
                        # per-queue or fabric-wide
                        h = gb // 2
                        for g in range(ngrp):
                            xnr = xn_pool.tile([P, gb, INUM], F32, tag="xr")
                            if split_q:
                                nc.sync.dma_start(out=xnr[:, 0:h, :],
                                                  in_=x_v[g][:, 0:h, :])
                                nc.scalar.dma_start(out=xnr[:, h:gb, :],
                                                    in_=x_v[g][:, h:gb, :])
                            else:
                                nc.sync.dma_start(out=xnr[:], in_=x_v[g])
                        return
                    for g in range(min(prefetch + 1, ngrp)):
                        emit_fetch(g)
                    for g in range(ngrp):
                        xt = xts.pop(g)
                        emit_compute(xt, g)
                        if g + prefetch + 1 < ngrp:
                            emit_fetch(g + prefetch + 1)

                def run_pe_only():
                    for g in range(ngrp):
                        emit_compute(xt_res, g)

                def emit_compute(xt, g):
                    ob = None
                    if True:
                        for j in range(gb):
                            bt = g * gb + j
                            jj = bt % store_gb
                            if jj == 0:
                                ob = ob_pool.tile([P, store_gb, ONUM], F32,
                                                  tag="ob")
                            ps = ps_pool.tile([P, ONUM], F32, tag="ps")
                            if fp8 and pe_t:
                                for t in range(NK // 2):
                                    nc.tensor.matmul(
                                        ps[:],
                                        xt[:, j * NK + 2 * t:
                                           j * NK + 2 * t + 2, :],
                                        wbt8p[:, 2 * t:2 * t + 2, :],
                                        start=(t == 0),
                                        stop=(t == NK // 2 - 1),
                                        perf_mode=mybir.MatmulPerfMode
                                        .DoubleRow)
                            elif fp8:
                                # DoubleRowSwInterleave: the stationary
                                # operand is the RAW pair-interleaved 256-byte
                                # block the 2-byte xbar transpose produced
                                # (walrus rejects byte-strided Ldweights APs;
                                # SWI is the hw mode for interleaved weights).
                                # The hw reads weight columns last-first, so
                                # psum partition m holds batch row 127-m; the
                                # host un-reverses each 128-row block.
                                xt_f8 = xt[:].bitcast(F8)  # [P, gb*NKP, 256]
                                for t in range(NKP):
                                    lhsT = xt_f8[:, j * NKP + t, :].rearrange(
                                        "p (a b) -> p a b", a=2)
                                    rhs = wbt8_f8[:, t, :].rearrange(
                                        "p (o e) -> p e o", e=2)
                                    nc.tensor.matmul(
                                        ps[:], lhsT, rhs,
                                        start=(t == 0), stop=(t == NKP - 1),
                                        perf_mode=mybir.MatmulPerfMode
                                        .DoubleRowSwInterleave)
                            else:
                                for k in range(NK):
                                    nc.tensor.matmul(ps[:],
                                                     xt[:, j * NK + k, :],
                                                     wbt[:, k, :],
                                                     start=(k == 0),
                                                     stop=(k == NK - 1))
                            nc.vector.tensor_copy(ob[:, jj, :], ps[:])
                            if jj == store_gb - 1:
                                nc.scalar.dma_start(out=o_v[bt // store_gb],
                                                    in_=ob[:])

                def emit_transpose(g, xn, xt):
                    if g in pe_groups:
                        # tensor-engine transpose: each [128b,128i]
                        # sub-tile via identity matmul into bf16 PSUM,
                        # then DVE copies a quad back to SBUF
                        for j in range(gb):
                            for q in range(NK // QT):
                                pst = pst_pool.tile([P, QT, P], BF16,
                                                    tag="pst")
                                for h in range(QT):
                                    k = q * QT + h
                                    nc.tensor.transpose(
                                        pst[:, h, :],
                                        xn[:, j, k * P:(k + 1) * P],
                                        ident[:])
                                kk = j * NK + q * QT
                                nc.vector.tensor_copy(
                                    xt[:, kk:kk + QT, :], pst[:])
                    else:
                        nc.sync.dma_start(out=xt[:], in_=xn[:],
                                          transpose=True)

                xt_res = None
                if mode == "pe":
                    # one-time fetch of a single xt tile the looped compute
                    # re-reads every "group"
                    if fp8:
                        xn8 = xn_pool.tile([P, gb, INUM], F8, tag="xn")
                        nc.gpsimd.dma_start(out=xn8[:], in_=x_v[0])
                        xt_res = xt_pool.tile([P, gb * NKP, P], BF16,
                                              tag="xt")
                        nc.sync.dma_start(out=xt_res[:],
                                          in_=xn8[:].bitcast(BF16),
                                          transpose=True)
                    else:
                        xn = xn_pool.tile([P, gb, INUM], BF16, tag="xn")
                        nc.gpsimd.dma_start(out=xn[:], in_=x_v[0])
                        xt_res = xt_pool.tile([P, gb * NK, P], BF16,
                                              tag="xt")
                        nc.sync.dma_start(out=xt_res[:], in_=xn[:],
                                          transpose=True)

                if loop is None:
                    run_groups()
                else:
                    with tc.For_i(0, loop, 1):
                        run_groups()
            _aux.close()   # LIFO: release pst/ident after xn/xt/ob

    nc.compile()
    return nc


def _make_exec(nc):
    """Build a jitted shard_map executable over the 8 cores (mirrors
    bass2jax.run_bass_via_pjrt's multi-core path, without donation so the
    same device buffers can be re-executed for timing)."""
    import jax
    from jax.sharding import Mesh, PartitionSpec
    from jax.experimental.shard_map import shard_map
    from concourse import bass2jax

    bass2jax.install_neuronx_cc_hook()
    partition_name = (nc.partition_id_tensor.name
                      if nc.partition_id_tensor else None)
    in_names, out_names, out_avals = [], [], []
    for alloc in nc.m.functions[0].allocations:
        if not isinstance(alloc, mybir.MemoryLocationSet):
            continue
        name = alloc.memorylocations[0].name
        if alloc.kind == "ExternalInput":
            if name != partition_name:
                in_names.append(name)
        elif alloc.kind == "ExternalOutput":
            out_names.append(name)
            out_avals.append(jax.core.ShapedArray(
                tuple(alloc.tensor_shape), mybir.dt.np(alloc.dtype)))
    n_params = len(in_names)
    all_names = in_names + out_names
    if partition_name is not None:
        all_names = all_names + [partition_name]

    def _body(*args):
        operands = list(args)
        if partition_name is not None:
            operands.append(bass2jax.partition_id_tensor())
        return tuple(bass2jax._bass_exec_p.bind(
            *operands,
            out_avals=tuple(out_avals),
            in_names=tuple(all_names),
            out_names=tuple(out_names),
            lowering_input_output_aliases=(),
            sim_require_finite=True,
            sim_require_nnan=True,
            nc=nc,
        ))

    devices = jax.devices()[:NCORES]
    mesh = Mesh(np.asarray(devices), ("core",))

    def make_fn(reps):
        def _rep_body(*args):
            outs = None
            for _ in range(reps):
                outs = _body(*args)   # effectful primitive: not CSE'd
            return outs
        return jax.jit(
            shard_map(_rep_body, mesh=mesh,
                      in_specs=(PartitionSpec("core"),) * (n_params + len(out_names)),
                      out_specs=(PartitionSpec("core"),) * len(out_names),
                      check_rep=False),
            keep_unused=True,
        )

    return make_fn, mesh, in_names[:n_params], out_names, out_avals


def bench(x, weight, u, r_lo=32, r_hi=1024, iters=6, **build_kw):
    """Measure real device time for one kernel execution.

    The axon RPC jitter (tens of ms) swamps a single ~250us execution, and
    multiple identical bass_exec calls in one program get CSE'd. So we
    build two NEFF variants whose phase B repeats in an on-device For_i
    loop (r_lo and r_hi iterations) and difference the wall-clock minima:
    (t_hi - t_lo)/(r_hi - r_lo) is one full phase-B pass of device time.
    Phase A (binarize+transpose weights, ~25us, runs once) is added from
    its cost-model share."""
    import time
    import jax
    from jax.sharding import NamedSharding, PartitionSpec

    concat = {
        "x": np.ascontiguousarray(x, dtype=np.float32),
        "weight": np.concatenate([weight] * NCORES, axis=0),
        "u": np.concatenate([u] * NCORES, axis=0),
    }

    def run_variant(r):
        nc = build(loop=r, **build_kw)
        make_fn, mesh, in_names, out_names, out_avals = _make_exec(nc)
        sh = NamedSharding(mesh, PartitionSpec("core"))
        args = [jax.device_put(concat[n], sh) for n in in_names]
        zeros = [
            jax.device_put(
                np.zeros((NCORES * a.shape[0], *a.shape[1:]), a.dtype), sh)
            for a in out_avals
        ]
        fn = make_fn(1)
        jax.block_until_ready(fn(*args, *zeros))    # compile + warm
        best = float("inf")
        for _ in range(iters):
            t0 = time.perf_counter()
            jax.block_until_ready(fn(*args, *zeros))
            best = min(best, time.perf_counter() - t0)
        return best

    t_lo = run_variant(r_lo)
    t_hi = run_variant(r_hi)
    pass_ns = (t_hi - t_lo) / (r_hi - r_lo) * 1e9
    phase_a_ns = 25_000.0   # one-time weight binarize+transpose (cost model)
    print(f"bench: loop{r_lo}={t_lo*1e3:.1f}ms loop{r_hi}={t_hi*1e3:.1f}ms "
          f"-> phase-B pass {pass_ns/1e3:.1f}us + phase-A ~{phase_a_ns/1e3:.0f}us")
    return pass_ns + phase_a_ns


def kernel(x, weight, u):
    x = np.ascontiguousarray(np.asarray(x), dtype=np.float32)
    weight = np.ascontiguousarray(np.asarray(weight), dtype=np.float32)
    u = np.ascontiguousarray(np.asarray(u), dtype=np.float32)
    assert x.shape == (B, INUM) and weight.shape == (ONUM, INUM)

    nc = _CACHE.get("nc")
    if nc is None:
        nc = _CACHE["nc"] = build(pe_t=DEFAULT_PE_T)

    in_maps = [
        {"x": x[c * BLOC:(c + 1) * BLOC], "weight": weight, "u": u}
        for c in range(NCORES)
    ]
    res = run_bass_kernel_spmd(nc, in_maps, list(range(NCORES)))
    out = np.concatenate([res.results[c]["out"] for c in range(NCORES)],
                         axis=0)
    if not DEFAULT_PE_T:
        # SWI matmuls emit each 128-row batch tile with rows reversed
        out = np.ascontiguousarray(
            out.reshape(-1, P, ONUM)[:, ::-1, :].reshape(B, ONUM))
    return out



# revision 40
# speedup vs baseline: 1.4489x; 1.0478x over previous
"""Trainium2 Bass kernel: BinarizedLinear  out = x @ (u < weight).T

Shapes (hardcoded): x [16384, 4096] f32, weight/u [512, 4096] f32,
out [16384, 512] f32.

Sharding: data-parallel over 8 NeuronCores — x sharded along batch
(2048 rows/core), weight/u replicated, no collectives; host concatenates
the per-core outputs.

Per-core kernel (Tile framework), default pe_t=True design:
  Phase A: load weight/u (fp32), binarize on DVE (u < weight -> bf16
           {0,1}), xbar-DMA-transpose so the contraction dim lands on
           partitions, cast to plane-packed fp8e4 wbt8p[i_loc, k, o]
           (resident, 2 MB).
  Phase B: per 2-batch-tile group: SWDGE cast-load x fp32->fp8e4
           (read-side HBM bound, ~330 GB/s/core), transpose on the
           TENSOR engine (identity matmul -> fp8 PSUM -> DVE/ACT copy
           to SBUF) keeping the shared DMA bus free of xbar traffic,
           then fp8 DoubleRow matmuls (two 128-k-tiles per instruction,
           fp32 PSUM), DVE copy PSUM->SBUF, store.

The alternate pe_t=False path transposes fp8 data on the DMA xbar in
2-byte units (a (2q,2q+1) fp8 pair per unit) and consumes the
pair-interleaved block directly as DoubleRowSwInterleave stationary
operands; the hw reads weight columns last-first, so each 128-row
batch tile returns row-reversed and the host unreverses. Measured on
hw: the xbar path spends ~62us/pass of shared-bus time on transposes,
the pe_t path moves that to PE idle time.

fp8e4 ({0,1} weights are exact; x in [0,1) quantizes at ~3% RMS per
element, averaging out to ~6e-4 relative on the 4096-term dot product,
well inside the 2e-2 gate) doubles PE matmul throughput vs bf16.
"""

import numpy as np

from concourse import bass, bacc, mybir, tile
from concourse.bass_utils import run_bass_kernel_spmd

B, INUM, ONUM = 16384, 4096, 512
NCORES = 8
BLOC = B // NCORES  # 2048 batch rows per core
P = 128             # partitions
NK = INUM // P      # 32 contraction tiles
NOT = ONUM // P     # 4 weight-row tiles

F32 = mybir.dt.float32
BF16 = mybir.dt.bfloat16
F8 = mybir.dt.float8e4

# default transpose strategy: pe_t=True moves the x transposes off the
# shared DMA bus onto the tensor engine (plain DoubleRow, no row
# reversal); pe_t=False is the xbar/SwInterleave path whose 128-row
# batch tiles come back row-reversed (host unreverses).
DEFAULT_PE_T = True

_CACHE = {}


def build(bloc=BLOC, gb=2, xn_bufs=3, xt_bufs=4, ob_bufs=4, ps_bufs=8,
          store_gb=1, loop=None, pe_groups=(), fp8=True, mode="full",
          pe_t=None, split_q=False, stag=False, unroll=1):
    """gb: batch tiles (of 128 rows) grouped per x load/transpose DMA.

    loop: if set, wrap phase B in a For_i repeating it `loop` times
    (timing variant: same data each iteration, outputs overwritten).

    pe_groups: group indices whose x-transpose runs on the tensor engine
    (identity matmul -> bf16 PSUM -> DVE copy) instead of the DMA xbar,
    rebalancing SDMA-engine work onto PE idle time.

    fp8: 2-byte-unit xbar transpose of fp8 data + DoubleRowSwInterleave
    matmuls (2 k-tiles per instruction); output batch tiles come back
    row-reversed (host un-reverses).

    mode: timing-variant scopes for component benches — "full" (default),
    "dma" (x loads + transposes only), "pe" (matmuls/copies/stores from
    one resident xt tile, no per-pass DMA in the loop)."""
    if pe_t is None:
        pe_t = DEFAULT_PE_T
    nbt = bloc // P
    ngrp = nbt // gb
    NKP = INUM // 256    # fp8 paired k'-tiles (256 contraction per matmul)
    nc = bacc.Bacc("TRN2", target_bir_lowering=False, debug=False,
                   num_devices=NCORES)
    x_d = nc.dram_tensor("x", [bloc, INUM], F32, kind="ExternalInput")
    w_d = nc.dram_tensor("weight", [ONUM, INUM], F32, kind="ExternalInput")
    u_d = nc.dram_tensor("u", [ONUM, INUM], F32, kind="ExternalInput")
    o_d = nc.dram_tensor("out", [bloc, ONUM], F32, kind="ExternalOutput")

    # DRAM views, partition-major: x_v[g][p, j, i] = x[(g*gb + j)*P + p, i]
    x_v = x_d[:, :].rearrange("(g j p) i -> g p j i", g=ngrp, j=gb, p=P)
    o_v = o_d[:, :].rearrange("(g j p) o -> g p j o", g=nbt // store_gb,
                              j=store_gb, p=P)

    if pe_groups or pe_t:
        ps_bufs = min(ps_bufs, 6)   # leave 2 PSUM banks for PE transposes

    with tile.TileContext(nc) as tc:
        with (
            tc.tile_pool(name="wbt", bufs=1) as wbt_pool,
            tc.tile_pool(name="ps", bufs=ps_bufs, space="PSUM") as ps_pool,
        ):
            from contextlib import ExitStack
            _aux = ExitStack()
            ident = ident8 = None
            if pe_groups:
                from concourse import masks
                ident_pool = _aux.enter_context(
                    tc.tile_pool(name="ident", bufs=1))
                ident = ident_pool.tile([P, P], BF16)
                masks.make_identity(nc, ident[:])
            if fp8 and pe_t:
                from concourse import masks
                ident_pool = _aux.enter_context(
                    tc.tile_pool(name="ident8", bufs=1))
                ident_b = ident_pool.tile([P, P], BF16)
                masks.make_identity(nc, ident_b[:])
                ident8 = ident_pool.tile([P, P], F8)
                nc.vector.tensor_copy(ident8[:], ident_b[:])
            # ---- Phase A: binarized, transposed weights (resident) ----
            # bf16 path: wbt[i_local, k, o] = (u < weight)[o, k*128 + i]
            # fp8 path:  the binarized row is cast to fp8e4 and the xbar
            #   transpose runs on a bf16 *view*, so each transposed 2-byte
            #   unit carries the (i=2q, i=2q+1) fp8 pair; the pair becomes
            #   DoubleRow's two k-planes via byte-strided APs. wbt8_bf
            #   [u, k', o] (bf16 units) = fp8 bytes wb[o, k'*256 + 2u + e].
            # One 2MB DMA per o-tile per tensor (chunking these into 32
            # small DMAs put ~80us of serialized SWDGE latency on the
            # critical path gating the first matmul); w rides SWDGE while
            # u rides the scalar HWDGE so descriptor generation overlaps.
            # wu/wb pools are phase-A-scoped; SBUF is reclaimed for B.
            if fp8 and pe_t:
                # plane-packed fp8 weights for plain DoubleRow:
                # wbt8p[i_loc, k, o] = fp8(wb[o, k*128 + i_loc])
                wbt8p = wbt_pool.tile([P, NK, ONUM], F8)
            elif fp8:
                wbt8_bf = wbt_pool.tile([P, NKP, ONUM], BF16)
                wbt8_f8 = wbt8_bf[:].bitcast(F8)   # [P, NKP, 2*ONUM]
            else:
                wbt = wbt_pool.tile([P, NK, ONUM], BF16)
            with (
                tc.tile_pool(name="wu", bufs=2) as wu_pool,
                tc.tile_pool(name="wb", bufs=2) as wb_pool,
            ):
                for ot in range(NOT):
                    w_t = wu_pool.tile([P, INUM], F32, tag="w")
                    u_t = wu_pool.tile([P, INUM], F32, tag="u")
                    nc.gpsimd.dma_start(out=w_t[:],
                                        in_=w_d[ot * P:(ot + 1) * P, :])
                    nc.scalar.dma_start(out=u_t[:],
                                        in_=u_d[ot * P:(ot + 1) * P, :])
                    if fp8 and pe_t:
                        wb_t = wb_pool.tile([P, INUM], BF16, tag="wb")
                        nc.vector.tensor_tensor(wb_t[:], u_t[:], w_t[:],
                                                op=mybir.AluOpType.is_lt)
                        wt_t = wb_pool.tile([P, NK, P], BF16, tag="wt")
                        nc.sync.dma_start(out=wt_t[:], in_=wb_t[:],
                                          transpose=True)
                        cast = (nc.vector.tensor_copy if ot % 2 == 0
                                else nc.scalar.copy)
                        cast(wbt8p[:, :, ot * P:(ot + 1) * P], wt_t[:])
                    elif fp8:
                        wb8_t = wb_pool.tile([P, INUM], F8, tag="wb8")
                        nc.vector.tensor_tensor(wb8_t[:], u_t[:], w_t[:],
                                                op=mybir.AluOpType.is_lt)
                        nc.sync.dma_start(
                            out=wbt8_bf[:, :, ot * P:(ot + 1) * P],
                            in_=wb8_t[:].bitcast(BF16), transpose=True)
                    else:
                        wb_t = wb_pool.tile([P, INUM], BF16, tag="wb")
                        nc.vector.tensor_tensor(wb_t[:], u_t[:], w_t[:],
                                                op=mybir.AluOpType.is_lt)
                        nc.sync.dma_start(
                            out=wbt[:, :, ot * P:(ot + 1) * P],
                            in_=wb_t[:], transpose=True)

            # ---- Phase B: stream batch tiles, gb tiles per DMA group ----
            with (
                tc.tile_pool(name="xn", bufs=xn_bufs) as xn_pool,
                tc.tile_pool(name="xt", bufs=xt_bufs) as xt_pool,
                tc.tile_pool(name="ob", bufs=ob_bufs) as ob_pool,
            ):
                pst_pool = None
                if pe_groups or (fp8 and pe_t):
                    pst_pool = _aux.enter_context(
                        tc.tile_pool(name="pst", bufs=2, space="PSUM"))
                QT = 4   # k-tiles per PE-transpose PSUM staging quad

                def emit_load(xn8, g):
                    # casting DMAs are SWDGE-only, so the fp8 cast-load
                    # always rides the gpsimd queue
                    nc.gpsimd.dma_start(out=xn8[:], in_=x_v[g])

                def run_groups(_iv=None, prefetch=2):
                    # software-pipelined emission: group g+prefetch's
                    # load+transpose are emitted before group g's matmuls,
                    # so the prefetch DMAs outrank compute in the
                    # scheduler's priority order
                    xts = {}
                    if mode == "pe":
                        run_pe_only()
                        return

                    def emit_fetch(g):
                        if fp8 and pe_t:
                            # fp8 cast-load, then elementwise PE transposes
                            # (identity matmul, fp8 PSUM) + DVE/ACT copies
                            # to plane-packed xt8 — keeps the shared DMA bus
                            # free of the 2x xbar traffic.
                            xn8 = xn_pool.tile([P, gb, INUM], F8, tag="xn")
                            emit_load(xn8, g)
                            xt = xt_pool.tile([P, gb * NK, P], F8, tag="xt")
                            for j in range(gb):
                                for q in range(NK // QT):
                                    # fp8 PE transpose writes PSUM with an
                                    # element step of 2 bytes: stage in a
                                    # bf16 tile addressed via a stride-2
                                    # fp8 view
                                    pst = pst_pool.tile([P, QT, P], BF16,
                                                        tag="pst")
                                    pv = pst[:].bitcast(F8).rearrange(
                                        "p q (b e) -> p q e b", e=2)
                                    for h in range(QT):
                                        k = q * QT + h
                                        nc.tensor.transpose(
                                            pv[:, h, 0, :],
                                            xn8[:, j, k * P:(k + 1) * P],
                                            ident8[:])
                                    kk = j * NK + q * QT
                                    copy = (nc.vector.tensor_copy
                                            if q % 2 == 0 else nc.scalar.copy)
                                    copy(xt[:, kk:kk + QT, :], pv[:, :, 0, :])
                        elif fp8:
                            # xn8[p, j, i] = fp8(x[(g*gb + j)*P + p, i]),
                            # cast in the DMA; the xbar transpose then moves
                            # 2-byte units, i.e. (2q, 2q+1) fp8 pairs:
                            # xt8_bf[u, j*NKP + t, b] unit = fp8 bytes
                            # x[(g*gb+j)*P + b, t*256 + 2u + e]
                            xn8 = xn_pool.tile([P, gb, INUM], F8, tag="xn")
                            nc.gpsimd.dma_start(out=xn8[:], in_=x_v[g])
                            xt = xt_pool.tile([P, gb * NKP, P], BF16,
                                              tag="xt")
                            nc.sync.dma_start(out=xt[:],
                                              in_=xn8[:].bitcast(BF16),
                                              transpose=True)
                        else:
                            # xn[p, j, i] = x[(g*gb + j)*P + p, i]
                            xn = xn_pool.tile([P, gb, INUM], BF16, tag="xn")
                            nc.gpsimd.dma_start(out=xn[:], in_=x_v[g])
                            # xt[p, j*NK + k, f] = xn_2d[f, j*INUM + k*P + p]
                            #                    = x[(g*gb + j)*P + f, k*P+p]
                            xt = xt_pool.tile([P, gb * NK, P], BF16,
                                              tag="xt")
                            emit_transpose(g, xn, xt)
                        xts[g] = xt

                    ob = None
                    if mode == "dma":
                        for g in range(ngrp):
                            emit_fetch(g)
                        return
                    if mode == "load":
                        for g in range(ngrp):
                            xn8 = xn_pool.tile([P, gb, INUM],
                                               F8 if fp8 else BF16, tag="xn")
                            emit_load(xn8, g)
                        return
                    if mode == "loadraw":
                        # raw fp32 loads (no cast) split across two HWDGE
                        # queues — probes whether the load ceiling is
                        # per-queue or fabric-wide
                        h = gb // 2
                        for g in range(ngrp):
                            xnr = xn_pool.tile([P, gb, INUM], F32, tag="xr")
                            if split_q:
                                nc.sync.dma_start(out=xnr[:, 0:h, :],
                                                  in_=x_v[g][:, 0:h, :])
                                nc.scalar.dma_start(out=xnr[:, h:gb, :],
                                                    in_=x_v[g][:, h:gb, :])
                            else:
                                nc.sync.dma_start(out=xnr[:], in_=x_v[g])
                        return
                    for g in range(min(prefetch + 1, ngrp)):
                        emit_fetch(g)
                    for g in range(ngrp):
                        xt = xts.pop(g)
                        emit_compute(xt, g)
                        if g + prefetch + 1 < ngrp:
                            emit_fetch(g + prefetch + 1)

                def run_pe_only():
                    for g in range(ngrp):
                        emit_compute(xt_res, g)

                def emit_compute(xt, g):
                    ob = None
                    if True:
                        for j in range(gb):
                            bt = g * gb + j
                            jj = bt % store_gb
                            if jj == 0:
                                ob = ob_pool.tile([P, store_gb, ONUM], F32,
                                                  tag="ob")
                            ps = ps_pool.tile([P, ONUM], F32, tag="ps")
                            if fp8 and pe_t:
                                for t in range(NK // 2):
                                    nc.tensor.matmul(
                                        ps[:],
                                        xt[:, j * NK + 2 * t:
                                           j * NK + 2 * t + 2, :],
                                        wbt8p[:, 2 * t:2 * t + 2, :],
                                        start=(t == 0),
                                        stop=(t == NK // 2 - 1),
                                        perf_mode=mybir.MatmulPerfMode
                                        .DoubleRow)
                            elif fp8:
                                # DoubleRowSwInterleave: the stationary
                                # operand is the RAW pair-interleaved 256-byte
                                # block the 2-byte xbar transpose produced
                                # (walrus rejects byte-strided Ldweights APs;
                                # SWI is the hw mode for interleaved weights).
                                # The hw reads weight columns last-first, so
                                # psum partition m holds batch row 127-m; the
                                # host un-reverses each 128-row block.
                                xt_f8 = xt[:].bitcast(F8)  # [P, gb*NKP, 256]
                                for t in range(NKP):
                                    lhsT = xt_f8[:, j * NKP + t, :].rearrange(
                                        "p (a b) -> p a b", a=2)
                                    rhs = wbt8_f8[:, t, :].rearrange(
                                        "p (o e) -> p e o", e=2)
                                    nc.tensor.matmul(
                                        ps[:], lhsT, rhs,
                                        start=(t == 0), stop=(t == NKP - 1),
                                        perf_mode=mybir.MatmulPerfMode
                                        .DoubleRowSwInterleave)
                            else:
                                for k in range(NK):
                                    nc.tensor.matmul(ps[:],
                                                     xt[:, j * NK + k, :],
                                                     wbt[:, k, :],
                                                     start=(k == 0),
                                                     stop=(k == NK - 1))
                            nc.vector.tensor_copy(ob[:, jj, :], ps[:])
                            if jj == store_gb - 1:
                                nc.scalar.dma_start(out=o_v[bt // store_gb],
                                                    in_=ob[:])

                def emit_transpose(g, xn, xt):
                    if g in pe_groups:
                        # tensor-engine transpose: each [128b,128i]
                        # sub-tile via identity matmul into bf16 PSUM,
                        # then DVE copies a quad back to SBUF
                        for j in range(gb):
                            for q in range(NK // QT):
                                pst = pst_pool.tile([P, QT, P], BF16,
                                                    tag="pst")
                                for h in range(QT):
                                    k = q * QT + h
                                    nc.tensor.transpose(
                                        pst[:, h, :],
                                        xn[:, j, k * P:(k + 1) * P],
                                        ident[:])
                                kk = j * NK + q * QT
                                nc.vector.tensor_copy(
                                    xt[:, kk:kk + QT, :], pst[:])
                    else:
                        nc.sync.dma_start(out=xt[:], in_=xn[:],
                                          transpose=True)

                xt_res = None
                if mode == "pe":
                    # one-time fetch of a single xt tile the looped compute
                    # re-reads every "group"
                    if fp8:
                        xn8 = xn_pool.tile([P, gb, INUM], F8, tag="xn")
                        nc.gpsimd.dma_start(out=xn8[:], in_=x_v[0])
                        xt_res = xt_pool.tile([P, gb * NKP, P], BF16,
                                              tag="xt")
                        nc.sync.dma_start(out=xt_res[:],
                                          in_=xn8[:].bitcast(BF16),
                                          transpose=True)
                    else:
                        xn = xn_pool.tile([P, gb, INUM], BF16, tag="xn")
                        nc.gpsimd.dma_start(out=xn[:], in_=x_v[0])
                        xt_res = xt_pool.tile([P, gb * NK, P], BF16,
                                              tag="xt")
                        nc.sync.dma_start(out=xt_res[:], in_=xn[:],
                                          transpose=True)

                if loop is None:
                    run_groups()
                else:
                    # unroll>1 places several passes in one loop body so
                    # consecutive passes overlap without the For_i
                    # all-engine barrier between them
                    assert loop % unroll == 0
                    with tc.For_i(0, loop // unroll, 1,
                                  staggered_reset=stag):
                        for _ in range(unroll):
                            run_groups()
            _aux.close()   # LIFO: release pst/ident after xn/xt/ob

    nc.compile()
    return nc


def _make_exec(nc):
    """Build a jitted shard_map executable over the 8 cores (mirrors
    bass2jax.run_bass_via_pjrt's multi-core path, without donation so the
    same device buffers can be re-executed for timing)."""
    import jax
    from jax.sharding import Mesh, PartitionSpec
    from jax.experimental.shard_map import shard_map
    from concourse import bass2jax

    bass2jax.install_neuronx_cc_hook()
    partition_name = (nc.partition_id_tensor.name
                      if nc.partition_id_tensor else None)
    in_names, out_names, out_avals = [], [], []
    for alloc in nc.m.functions[0].allocations:
        if not isinstance(alloc, mybir.MemoryLocationSet):
            continue
        name = alloc.memorylocations[0].name
        if alloc.kind == "ExternalInput":
            if name != partition_name:
                in_names.append(name)
        elif alloc.kind == "ExternalOutput":
            out_names.append(name)
            out_avals.append(jax.core.ShapedArray(
                tuple(alloc.tensor_shape), mybir.dt.np(alloc.dtype)))
    n_params = len(in_names)
    all_names = in_names + out_names
    if partition_name is not None:
        all_names = all_names + [partition_name]

    def _body(*args):
        operands = list(args)
        if partition_name is not None:
            operands.append(bass2jax.partition_id_tensor())
        return tuple(bass2jax._bass_exec_p.bind(
            *operands,
            out_avals=tuple(out_avals),
            in_names=tuple(all_names),
            out_names=tuple(out_names),
            lowering_input_output_aliases=(),
            sim_require_finite=True,
            sim_require_nnan=True,
            nc=nc,
        ))

    devices = jax.devices()[:NCORES]
    mesh = Mesh(np.asarray(devices), ("core",))

    def make_fn(reps):
        def _rep_body(*args):
            outs = None
            for _ in range(reps):
                outs = _body(*args)   # effectful primitive: not CSE'd
            return outs
        return jax.jit(
            shard_map(_rep_body, mesh=mesh,
                      in_specs=(PartitionSpec("core"),) * (n_params + len(out_names)),
                      out_specs=(PartitionSpec("core"),) * len(out_names),
                      check_rep=False),
            keep_unused=True,
        )

    return make_fn, mesh, in_names[:n_params], out_names, out_avals


def bench(x, weight, u, r_lo=32, r_hi=1024, iters=6, unroll=8, stag=True,
          **build_kw):
    """Measure real device time for one kernel execution.

    The axon RPC jitter (tens of ms) swamps a single ~250us execution, and
    multiple identical bass_exec calls in one program get CSE'd. So we
    build two NEFF variants whose phase B repeats in an on-device For_i
    loop (r_lo and r_hi iterations) and difference the wall-clock minima:
    (t_hi - t_lo)/(r_hi - r_lo) is one full phase-B pass of device time.
    Phase A (binarize+transpose weights, ~25us, runs once) is added from
    its cost-model share."""
    import time
    import jax
    from jax.sharding import NamedSharding, PartitionSpec

    concat = {
        "x": np.ascontiguousarray(x, dtype=np.float32),
        "weight": np.concatenate([weight] * NCORES, axis=0),
        "u": np.concatenate([u] * NCORES, axis=0),
    }

    def run_variant(r):
        nc = build(loop=r, unroll=unroll, stag=stag, **build_kw)
        make_fn, mesh, in_names, out_names, out_avals = _make_exec(nc)
        sh = NamedSharding(mesh, PartitionSpec("core"))
        args = [jax.device_put(concat[n], sh) for n in in_names]
        zeros = [
            jax.device_put(
                np.zeros((NCORES * a.shape[0], *a.shape[1:]), a.dtype), sh)
            for a in out_avals
        ]
        fn = make_fn(1)
        jax.block_until_ready(fn(*args, *zeros))    # compile + warm
        best = float("inf")
        for _ in range(iters):
            t0 = time.perf_counter()
            jax.block_until_ready(fn(*args, *zeros))
            best = min(best, time.perf_counter() - t0)
        return best

    t_lo = run_variant(r_lo)
    t_hi = run_variant(r_hi)
    pass_ns = (t_hi - t_lo) / (r_hi - r_lo) * 1e9
    phase_a_ns = 25_000.0   # one-time weight binarize+transpose (cost model)
    print(f"bench: loop{r_lo}={t_lo*1e3:.1f}ms loop{r_hi}={t_hi*1e3:.1f}ms "
          f"-> phase-B pass {pass_ns/1e3:.1f}us + phase-A ~{phase_a_ns/1e3:.0f}us")
    return pass_ns + phase_a_ns


def kernel(x, weight, u):
    x = np.ascontiguousarray(np.asarray(x), dtype=np.float32)
    weight = np.ascontiguousarray(np.asarray(weight), dtype=np.float32)
    u = np.ascontiguousarray(np.asarray(u), dtype=np.float32)
    assert x.shape == (B, INUM) and weight.shape == (ONUM, INUM)

    nc = _CACHE.get("nc")
    if nc is None:
        nc = _CACHE["nc"] = build(pe_t=DEFAULT_PE_T)

    in_maps = [
        {"x": x[c * BLOC:(c + 1) * BLOC], "weight": weight, "u": u}
        for c in range(NCORES)
    ]
    res = run_bass_kernel_spmd(nc, in_maps, list(range(NCORES)))
    out = np.concatenate([res.results[c]["out"] for c in range(NCORES)],
                         axis=0)
    if not DEFAULT_PE_T:
        # SWI matmuls emit each 128-row batch tile with rows reversed
        out = np.ascontiguousarray(
            out.reshape(-1, P, ONUM)[:, ::-1, :].reshape(B, ONUM))
    return out



# revision 53
# speedup vs baseline: 1.5038x; 1.0378x over previous
"""Trainium2 Bass kernel: BinarizedLinear  out = x @ (u < weight).T

Shapes (hardcoded): x [16384, 4096] f32, weight/u [512, 4096] f32,
out [16384, 512] f32.

Sharding: data-parallel over 8 NeuronCores — x sharded along batch
(2048 rows/core), weight/u replicated, no collectives; host concatenates
the per-core outputs.

Per-core kernel (Tile framework), default pe_t=True design:
  Phase A: load weight/u (fp32), binarize on DVE (u < weight -> bf16
           {0,1}), xbar-DMA-transpose so the contraction dim lands on
           partitions, cast to plane-packed fp8e4 wbt8p[i_loc, k, o]
           (resident, 2 MB).
  Phase B: per 2-batch-tile group: SWDGE cast-load x fp32->fp8e4
           (read-side HBM bound, ~330 GB/s/core), transpose on the
           TENSOR engine (identity matmul -> fp8 PSUM -> DVE/ACT copy
           to SBUF) keeping the shared DMA bus free of xbar traffic,
           then fp8 DoubleRow matmuls (two 128-k-tiles per instruction,
           fp32 PSUM), DVE copy PSUM->SBUF, store.

The alternate pe_t=False path transposes fp8 data on the DMA xbar in
2-byte units (a (2q,2q+1) fp8 pair per unit) and consumes the
pair-interleaved block directly as DoubleRowSwInterleave stationary
operands; the hw reads weight columns last-first, so each 128-row
batch tile returns row-reversed and the host unreverses. Measured on
hw: the xbar path spends ~62us/pass of shared-bus time on transposes,
the pe_t path moves that to PE idle time.

fp8e4 ({0,1} weights are exact; x in [0,1) quantizes at ~3% RMS per
element, averaging out to ~6e-4 relative on the 4096-term dot product,
well inside the 2e-2 gate) doubles PE matmul throughput vs bf16.
"""

import numpy as np

from concourse import bass, bacc, mybir, tile
from concourse.bass_utils import run_bass_kernel_spmd

B, INUM, ONUM = 16384, 4096, 512
NCORES = 8
BLOC = B // NCORES  # 2048 batch rows per core
P = 128             # partitions
NK = INUM // P      # 32 contraction tiles
NOT = ONUM // P     # 4 weight-row tiles

F32 = mybir.dt.float32
BF16 = mybir.dt.bfloat16
F8 = mybir.dt.float8e4
F16 = mybir.dt.float16

# default transpose strategy: pe_t=True moves the x transposes off the
# shared DMA bus onto the tensor engine (plain DoubleRow, no row
# reversal); pe_t=False is the xbar/SwInterleave path whose 128-row
# batch tiles come back row-reversed (host unreverses).
DEFAULT_PE_T = True
# packout: device writes a partition-major packed out scratch
# (out[p, bt, o] = canonical out[bt*128+p, o]) so batched stores emit
# store_gb*2KB contiguous descriptors per partition; host untangles.
DEFAULT_PACKOUT = False
# out16: device stores fp16 output (halves store bytes on the shared
# fabric; |out| < 1200 << 65504, rounding adds ~3e-4 rel), host upcasts.
DEFAULT_OUT16 = True

_CACHE = {}


def build(bloc=BLOC, gb=2, xn_bufs=3, xt_bufs=4, ob_bufs=4, ps_bufs=8,
          store_gb=1, loop=None, pe_groups=(), fp8=True, mode="full",
          pe_t=None, split_q=False, stag=False, unroll=1, prefetch=2,
          packout=None, out16=None):
    """gb: batch tiles (of 128 rows) grouped per x load/transpose DMA.

    loop: if set, wrap phase B in a For_i repeating it `loop` times
    (timing variant: same data each iteration, outputs overwritten).

    pe_groups: group indices whose x-transpose runs on the tensor engine
    (identity matmul -> bf16 PSUM -> DVE copy) instead of the DMA xbar,
    rebalancing SDMA-engine work onto PE idle time.

    fp8: 2-byte-unit xbar transpose of fp8 data + DoubleRowSwInterleave
    matmuls (2 k-tiles per instruction); output batch tiles come back
    row-reversed (host un-reverses).

    mode: timing-variant scopes for component benches — "full" (default),
    "dma" (x loads + transposes only), "pe" (matmuls/copies/stores from
    one resident xt tile, no per-pass DMA in the loop)."""
    if pe_t is None:
        pe_t = DEFAULT_PE_T
    if packout is None:
        packout = DEFAULT_PACKOUT
    if out16 is None:
        out16 = DEFAULT_OUT16
    ODT = F16 if out16 else F32
    nbt = bloc // P
    ngrp = nbt // gb
    NKP = INUM // 256    # fp8 paired k'-tiles (256 contraction per matmul)
    nc = bacc.Bacc("TRN2", target_bir_lowering=False, debug=False,
                   num_devices=NCORES)
    x_d = nc.dram_tensor("x", [bloc, INUM], F32, kind="ExternalInput")
    w_d = nc.dram_tensor("weight", [ONUM, INUM], F32, kind="ExternalInput")
    u_d = nc.dram_tensor("u", [ONUM, INUM], F32, kind="ExternalInput")

    # DRAM views, partition-major: x_v[g][p, j, i] = x[(g*gb + j)*P + p, i]
    x_v = x_d[:, :].rearrange("(g j p) i -> g p j i", g=ngrp, j=gb, p=P)
    if packout:
        # partition-major packed output scratch: out[p, bt, o] =
        # canonical out[bt*128 + p, o]. A batched store then writes
        # store_gb*2KB CONTIGUOUS bytes per partition (vs 2KB row-granular
        # descriptors in the canonical layout); the host untangles.
        o_d = nc.dram_tensor("out", [P, nbt, ONUM], ODT,
                             kind="ExternalOutput")
        o_v = o_d[:, :, :].rearrange("p (g j) o -> g p j o",
                                     g=nbt // store_gb, j=store_gb)
    else:
        o_d = nc.dram_tensor("out", [bloc, ONUM], ODT,
                             kind="ExternalOutput")
        o_v = o_d[:, :].rearrange("(g j p) o -> g p j o", g=nbt // store_gb,
                                  j=store_gb, p=P)

    if pe_groups or pe_t:
        ps_bufs = min(ps_bufs, 6)   # leave 2 PSUM banks for PE transposes

    with tile.TileContext(nc) as tc:
        with (
            tc.tile_pool(name="wbt", bufs=1) as wbt_pool,
            tc.tile_pool(name="ps", bufs=ps_bufs, space="PSUM") as ps_pool,
        ):
            from contextlib import ExitStack
            _aux = ExitStack()
            ident = ident8 = None
            if pe_groups:
                from concourse import masks
                ident_pool = _aux.enter_context(
                    tc.tile_pool(name="ident", bufs=1))
                ident = ident_pool.tile([P, P], BF16)
                masks.make_identity(nc, ident[:])
            if fp8 and pe_t:
                from concourse import masks
                ident_pool = _aux.enter_context(
                    tc.tile_pool(name="ident8", bufs=1))
                ident_b = ident_pool.tile([P, P], BF16)
                masks.make_identity(nc, ident_b[:])
                ident8 = ident_pool.tile([P, P], F8)
                nc.vector.tensor_copy(ident8[:], ident_b[:])
            # ---- Phase A: binarized, transposed weights (resident) ----
            # bf16 path: wbt[i_local, k, o] = (u < weight)[o, k*128 + i]
            # fp8 path:  the binarized row is cast to fp8e4 and the xbar
            #   transpose runs on a bf16 *view*, so each transposed 2-byte
            #   unit carries the (i=2q, i=2q+1) fp8 pair; the pair becomes
            #   DoubleRow's two k-planes via byte-strided APs. wbt8_bf
            #   [u, k', o] (bf16 units) = fp8 bytes wb[o, k'*256 + 2u + e].
            # One 2MB DMA per o-tile per tensor (chunking these into 32
            # small DMAs put ~80us of serialized SWDGE latency on the
            # critical path gating the first matmul); w rides SWDGE while
            # u rides the scalar HWDGE so descriptor generation overlaps.
            # wu/wb pools are phase-A-scoped; SBUF is reclaimed for B.
            if fp8 and pe_t:
                # plane-packed fp8 weights for plain DoubleRow:
                # wbt8p[i_loc, k, o] = fp8(wb[o, k*128 + i_loc])
                wbt8p = wbt_pool.tile([P, NK, ONUM], F8)
            elif fp8:
                wbt8_bf = wbt_pool.tile([P, NKP, ONUM], BF16)
                wbt8_f8 = wbt8_bf[:].bitcast(F8)   # [P, NKP, 2*ONUM]
            else:
                wbt = wbt_pool.tile([P, NK, ONUM], BF16)
            with (
                tc.tile_pool(name="wu", bufs=2) as wu_pool,
                tc.tile_pool(name="wb", bufs=2) as wb_pool,
            ):
                for ot in range(NOT):
                    w_t = wu_pool.tile([P, INUM], F32, tag="w")
                    u_t = wu_pool.tile([P, INUM], F32, tag="u")
                    nc.gpsimd.dma_start(out=w_t[:],
                                        in_=w_d[ot * P:(ot + 1) * P, :])
                    nc.scalar.dma_start(out=u_t[:],
                                        in_=u_d[ot * P:(ot + 1) * P, :])
                    if fp8 and pe_t:
                        wb_t = wb_pool.tile([P, INUM], BF16, tag="wb")
                        nc.vector.tensor_tensor(wb_t[:], u_t[:], w_t[:],
                                                op=mybir.AluOpType.is_lt)
                        wt_t = wb_pool.tile([P, NK, P], BF16, tag="wt")
                        nc.sync.dma_start(out=wt_t[:], in_=wb_t[:],
                                          transpose=True)
                        cast = (nc.vector.tensor_copy if ot % 2 == 0
                                else nc.scalar.copy)
                        cast(wbt8p[:, :, ot * P:(ot + 1) * P], wt_t[:])
                    elif fp8:
                        wb8_t = wb_pool.tile([P, INUM], F8, tag="wb8")
                        nc.vector.tensor_tensor(wb8_t[:], u_t[:], w_t[:],
                                                op=mybir.AluOpType.is_lt)
                        nc.sync.dma_start(
                            out=wbt8_bf[:, :, ot * P:(ot + 1) * P],
                            in_=wb8_t[:].bitcast(BF16), transpose=True)
                    else:
                        wb_t = wb_pool.tile([P, INUM], BF16, tag="wb")
                        nc.vector.tensor_tensor(wb_t[:], u_t[:], w_t[:],
                                                op=mybir.AluOpType.is_lt)
                        nc.sync.dma_start(
                            out=wbt[:, :, ot * P:(ot + 1) * P],
                            in_=wb_t[:], transpose=True)

            # ---- Phase B: stream batch tiles, gb tiles per DMA group ----
            with (
                tc.tile_pool(name="xn", bufs=xn_bufs) as xn_pool,
                tc.tile_pool(name="xt", bufs=xt_bufs) as xt_pool,
                tc.tile_pool(name="ob", bufs=ob_bufs) as ob_pool,
            ):
                pst_pool = None
                if pe_groups or (fp8 and pe_t):
                    pst_pool = _aux.enter_context(
                        tc.tile_pool(name="pst", bufs=2, space="PSUM"))
                QT = 4   # k-tiles per PE-transpose PSUM staging quad

                def emit_load(xn8, g):
                    # casting DMAs are SWDGE-only, so the fp8 cast-load
                    # always rides the gpsimd queue
                    nc.gpsimd.dma_start(out=xn8[:], in_=x_v[g])

                def run_groups(_iv=None, prefetch=prefetch):
                    # software-pipelined emission: group g+prefetch's
                    # load+transpose are emitted before group g's matmuls,
                    # so the prefetch DMAs outrank compute in the
                    # scheduler's priority order
                    xts = {}
                    if mode == "pe":
                        run_pe_only()
                        return

                    def emit_fetch(g):
                        if fp8 and pe_t:
                            # fp8 cast-load, then elementwise PE transposes
                            # (identity matmul, fp8 PSUM) + DVE/ACT copies
                            # to plane-packed xt8 — keeps the shared DMA bus
                            # free of the 2x xbar traffic.
                            xn8 = xn_pool.tile([P, gb, INUM], F8, tag="xn")
                            emit_load(xn8, g)
                            xt = xt_pool.tile([P, gb * NK, P], F8, tag="xt")
                            for j in range(gb):
                                for q in range(NK // QT):
                                    # fp8 PE transpose writes PSUM with an
                                    # element step of 2 bytes: stage in a
                                    # bf16 tile addressed via a stride-2
                                    # fp8 view
                                    pst = pst_pool.tile([P, QT, P], BF16,
                                                        tag="pst")
                                    pv = pst[:].bitcast(F8).rearrange(
                                        "p q (b e) -> p q e b", e=2)
                                    for h in range(QT):
                                        k = q * QT + h
                                        nc.tensor.transpose(
                                            pv[:, h, 0, :],
                                            xn8[:, j, k * P:(k + 1) * P],
                                            ident8[:])
                                    kk = j * NK + q * QT
                                    copy = (nc.vector.tensor_copy
                                            if q % 2 == 0 else nc.scalar.copy)
                                    copy(xt[:, kk:kk + QT, :], pv[:, :, 0, :])
                        elif fp8:
                            # xn8[p, j, i] = fp8(x[(g*gb + j)*P + p, i]),
                            # cast in the DMA; the xbar transpose then moves
                            # 2-byte units, i.e. (2q, 2q+1) fp8 pairs:
                            # xt8_bf[u, j*NKP + t, b] unit = fp8 bytes
                            # x[(g*gb+j)*P + b, t*256 + 2u + e]
                            xn8 = xn_pool.tile([P, gb, INUM], F8, tag="xn")
                            nc.gpsimd.dma_start(out=xn8[:], in_=x_v[g])
                            xt = xt_pool.tile([P, gb * NKP, P], BF16,
                                              tag="xt")
                            nc.sync.dma_start(out=xt[:],
                                              in_=xn8[:].bitcast(BF16),
                                              transpose=True)
                        else:
                            # xn[p, j, i] = x[(g*gb + j)*P + p, i]
                            xn = xn_pool.tile([P, gb, INUM], BF16, tag="xn")
                            nc.gpsimd.dma_start(out=xn[:], in_=x_v[g])
                            # xt[p, j*NK + k, f] = xn_2d[f, j*INUM + k*P + p]
                            #                    = x[(g*gb + j)*P + f, k*P+p]
                            xt = xt_pool.tile([P, gb * NK, P], BF16,
                                              tag="xt")
                            emit_transpose(g, xn, xt)
                        xts[g] = xt

                    ob = None
                    if mode == "dma":
                        for g in range(ngrp):
                            emit_fetch(g)
                        return
                    if mode == "load":
                        for g in range(ngrp):
                            xn8 = xn_pool.tile([P, gb, INUM],
                                               F8 if fp8 else BF16, tag="xn")
                            emit_load(xn8, g)
                        return
                    if mode == "loadstore":
                        # loads + stores, no compute: measures whether HBM
                        # writes sum with reads on the fabric
                        for g in range(ngrp):
                            xn8 = xn_pool.tile([P, gb, INUM],
                                               F8 if fp8 else BF16, tag="xn")
                            emit_load(xn8, g)
                            for j in range(gb):
                                bt = g * gb + j
                                if bt % store_gb == 0:
                                    ob = ob_pool.tile(
                                        [P, store_gb, ONUM], ODT, tag="ob")
                                    nc.vector.memset(ob[:], 0.0)
                                if bt % store_gb == store_gb - 1:
                                    nc.scalar.dma_start(
                                        out=o_v[bt // store_gb], in_=ob[:])
                        return
                    if mode == "loadraw":
                        # raw fp32 loads (no cast) split across two HWDGE
                        # queues — probes whether the load ceiling is
                        # per-queue or fabric-wide
                        h = gb // 2
                        for g in range(ngrp):
                            xnr = xn_pool.tile([P, gb, INUM], F32, tag="xr")
                            if split_q:
                                nc.sync.dma_start(out=xnr[:, 0:h, :],
                                                  in_=x_v[g][:, 0:h, :])
                                nc.scalar.dma_start(out=xnr[:, h:gb, :],
                                                    in_=x_v[g][:, h:gb, :])
                            else:
                                nc.sync.dma_start(out=xnr[:], in_=x_v[g])
                        return
                    for g in range(min(prefetch + 1, ngrp)):
                        emit_fetch(g)
                    for g in range(ngrp):
                        xt = xts.pop(g)
                        emit_compute(xt, g)
                        if g + prefetch + 1 < ngrp:
                            emit_fetch(g + prefetch + 1)

                def run_pe_only():
                    for g in range(ngrp):
                        emit_compute(xt_res, g)

                _ob_holder = [None]

                def emit_compute(xt, g):
                    if True:
                        for j in range(gb):
                            bt = g * gb + j
                            jj = bt % store_gb
                            if jj == 0:
                                ob = ob_pool.tile([P, store_gb, ONUM],
                                                  ODT, tag="ob", name="ob")
                                _ob_holder[0] = ob
                            ob = _ob_holder[0]
                            ps = ps_pool.tile([P, ONUM], F32, tag="ps")
                            if fp8 and pe_t:
                                for t in range(NK // 2):
                                    nc.tensor.matmul(
                                        ps[:],
                                        xt[:, j * NK + 2 * t:
                                           j * NK + 2 * t + 2, :],
                                        wbt8p[:, 2 * t:2 * t + 2, :],
                                        start=(t == 0),
                                        stop=(t == NK // 2 - 1),
                                        perf_mode=mybir.MatmulPerfMode
                                        .DoubleRow)
                            elif fp8:
                                # DoubleRowSwInterleave: the stationary
                                # operand is the RAW pair-interleaved 256-byte
                                # block the 2-byte xbar transpose produced
                                # (walrus rejects byte-strided Ldweights APs;
                                # SWI is the hw mode for interleaved weights).
                                # The hw reads weight columns last-first, so
                                # psum partition m holds batch row 127-m; the
                                # host un-reverses each 128-row block.
                                xt_f8 = xt[:].bitcast(F8)  # [P, gb*NKP, 256]
                                for t in range(NKP):
                                    lhsT = xt_f8[:, j * NKP + t, :].rearrange(
                                        "p (a b) -> p a b", a=2)
                                    rhs = wbt8_f8[:, t, :].rearrange(
                                        "p (o e) -> p e o", e=2)
                                    nc.tensor.matmul(
                                        ps[:], lhsT, rhs,
                                        start=(t == 0), stop=(t == NKP - 1),
                                        perf_mode=mybir.MatmulPerfMode
                                        .DoubleRowSwInterleave)
                            else:
                                for k in range(NK):
                                    nc.tensor.matmul(ps[:],
                                                     xt[:, j * NK + k, :],
                                                     wbt[:, k, :],
                                                     start=(k == 0),
                                                     stop=(k == NK - 1))
                            nc.vector.tensor_copy(ob[:, jj, :], ps[:])
                            if jj == store_gb - 1:
                                nc.scalar.dma_start(out=o_v[bt // store_gb],
                                                    in_=ob[:])

                def emit_transpose(g, xn, xt):
                    if g in pe_groups:
                        # tensor-engine transpose: each [128b,128i]
                        # sub-tile via identity matmul into bf16 PSUM,
                        # then DVE copies a quad back to SBUF
                        for j in range(gb):
                            for q in range(NK // QT):
                                pst = pst_pool.tile([P, QT, P], BF16,
                                                    tag="pst")
                                for h in range(QT):
                                    k = q * QT + h
                                    nc.tensor.transpose(
                                        pst[:, h, :],
                                        xn[:, j, k * P:(k + 1) * P],
                                        ident[:])
                                kk = j * NK + q * QT
                                nc.vector.tensor_copy(
                                    xt[:, kk:kk + QT, :], pst[:])
                    else:
                        nc.sync.dma_start(out=xt[:], in_=xn[:],
                                          transpose=True)

                xt_res = None
                if mode == "pe":
                    # one-time fetch of a single xt tile the looped compute
                    # re-reads every "group"
                    if fp8:
                        xn8 = xn_pool.tile([P, gb, INUM], F8, tag="xn")
                        nc.gpsimd.dma_start(out=xn8[:], in_=x_v[0])
                        xt_res = xt_pool.tile([P, gb * NKP, P], BF16,
                                              tag="xt")
                        nc.sync.dma_start(out=xt_res[:],
                                          in_=xn8[:].bitcast(BF16),
                                          transpose=True)
                    else:
                        xn = xn_pool.tile([P, gb, INUM], BF16, tag="xn")
                        nc.gpsimd.dma_start(out=xn[:], in_=x_v[0])
                        xt_res = xt_pool.tile([P, gb * NK, P], BF16,
                                              tag="xt")
                        nc.sync.dma_start(out=xt_res[:], in_=xn[:],
                                          transpose=True)

                if loop is None:
                    run_groups()
                else:
                    # unroll>1 places several passes in one loop body so
                    # consecutive passes overlap without the For_i
                    # all-engine barrier between them
                    assert loop % unroll == 0
                    with tc.For_i(0, loop // unroll, 1,
                                  staggered_reset=stag):
                        for _ in range(unroll):
                            run_groups()
            _aux.close()   # LIFO: release pst/ident after xn/xt/ob

    nc.compile()
    return nc


def _make_exec(nc):
    """Build a jitted shard_map executable over the 8 cores (mirrors
    bass2jax.run_bass_via_pjrt's multi-core path, without donation so the
    same device buffers can be re-executed for timing)."""
    import jax
    from jax.sharding import Mesh, PartitionSpec
    from jax.experimental.shard_map import shard_map
    from concourse import bass2jax

    bass2jax.install_neuronx_cc_hook()
    partition_name = (nc.partition_id_tensor.name
                      if nc.partition_id_tensor else None)
    in_names, out_names, out_avals = [], [], []
    for alloc in nc.m.functions[0].allocations:
        if not isinstance(alloc, mybir.MemoryLocationSet):
            continue
        name = alloc.memorylocations[0].name
        if alloc.kind == "ExternalInput":
            if name != partition_name:
                in_names.append(name)
        elif alloc.kind == "ExternalOutput":
            out_names.append(name)
            out_avals.append(jax.core.ShapedArray(
                tuple(alloc.tensor_shape), mybir.dt.np(alloc.dtype)))
    n_params = len(in_names)
    all_names = in_names + out_names
    if partition_name is not None:
        all_names = all_names + [partition_name]

    def _body(*args):
        operands = list(args)
        if partition_name is not None:
            operands.append(bass2jax.partition_id_tensor())
        return tuple(bass2jax._bass_exec_p.bind(
            *operands,
            out_avals=tuple(out_avals),
            in_names=tuple(all_names),
            out_names=tuple(out_names),
            lowering_input_output_aliases=(),
            sim_require_finite=True,
            sim_require_nnan=True,
            nc=nc,
        ))

    devices = jax.devices()[:NCORES]
    mesh = Mesh(np.asarray(devices), ("core",))

    def make_fn(reps):
        def _rep_body(*args):
            outs = None
            for _ in range(reps):
                outs = _body(*args)   # effectful primitive: not CSE'd
            return outs
        return jax.jit(
            shard_map(_rep_body, mesh=mesh,
                      in_specs=(PartitionSpec("core"),) * (n_params + len(out_names)),
                      out_specs=(PartitionSpec("core"),) * len(out_names),
                      check_rep=False),
            keep_unused=True,
        )

    return make_fn, mesh, in_names[:n_params], out_names, out_avals


def bench(x, weight, u, r_lo=32, r_hi=1024, iters=6, unroll=8, stag=True,
          **build_kw):
    """Measure real device time for one kernel execution.

    The axon RPC jitter (tens of ms) swamps a single ~250us execution, and
    multiple identical bass_exec calls in one program get CSE'd. So we
    build two NEFF variants whose phase B repeats in an on-device For_i
    loop (r_lo and r_hi iterations) and difference the wall-clock minima:
    (t_hi - t_lo)/(r_hi - r_lo) is one full phase-B pass of device time.
    Phase A (binarize+transpose weights, ~25us, runs once) is added from
    its cost-model share."""
    import time
    import jax
    from jax.sharding import NamedSharding, PartitionSpec

    concat = {
        "x": np.ascontiguousarray(x, dtype=np.float32),
        "weight": np.concatenate([weight] * NCORES, axis=0),
        "u": np.concatenate([u] * NCORES, axis=0),
    }

    def run_variant(r):
        nc = build(loop=r, unroll=unroll, stag=stag, **build_kw)
        make_fn, mesh, in_names, out_names, out_avals = _make_exec(nc)
        sh = NamedSharding(mesh, PartitionSpec("core"))
        args = [jax.device_put(concat[n], sh) for n in in_names]
        zeros = [
            jax.device_put(
                np.zeros((NCORES * a.shape[0], *a.shape[1:]), a.dtype), sh)
            for a in out_avals
        ]
        fn = make_fn(1)
        jax.block_until_ready(fn(*args, *zeros))    # compile + warm
        best = float("inf")
        for _ in range(iters):
            t0 = time.perf_counter()
            jax.block_until_ready(fn(*args, *zeros))
            best = min(best, time.perf_counter() - t0)
        return best

    t_lo = run_variant(r_lo)
    t_hi = run_variant(r_hi)
    pass_ns = (t_hi - t_lo) / (r_hi - r_lo) * 1e9
    phase_a_ns = 25_000.0   # one-time weight binarize+transpose (cost model)
    print(f"bench: loop{r_lo}={t_lo*1e3:.1f}ms loop{r_hi}={t_hi*1e3:.1f}ms "
          f"-> phase-B pass {pass_ns/1e3:.1f}us + phase-A ~{phase_a_ns/1e3:.0f}us")
    return pass_ns + phase_a_ns


def kernel(x, weight, u):
    x = np.ascontiguousarray(np.asarray(x), dtype=np.float32)
    weight = np.ascontiguousarray(np.asarray(weight), dtype=np.float32)
    u = np.ascontiguousarray(np.asarray(u), dtype=np.float32)
    assert x.shape == (B, INUM) and weight.shape == (ONUM, INUM)

    nc = _CACHE.get("nc")
    if nc is None:
        nc = _CACHE["nc"] = build(pe_t=DEFAULT_PE_T)

    in_maps = [
        {"x": x[c * BLOC:(c + 1) * BLOC], "weight": weight, "u": u}
        for c in range(NCORES)
    ]
    res = run_bass_kernel_spmd(nc, in_maps, list(range(NCORES)))
    if DEFAULT_PACKOUT:
        # untangle the partition-major packed scratch per core
        out = np.concatenate(
            [np.asarray(res.results[c]["out"]).transpose(1, 0, 2)
             .reshape(BLOC, ONUM) for c in range(NCORES)], axis=0)
    else:
        out = np.concatenate([res.results[c]["out"] for c in range(NCORES)],
                             axis=0)
    if DEFAULT_OUT16:
        out = out.astype(np.float32)
    if not DEFAULT_PE_T:
        # SWI matmuls emit each 128-row batch tile with rows reversed
        out = np.ascontiguousarray(
            out.reshape(-1, P, ONUM)[:, ::-1, :].reshape(B, ONUM))
    return out

